# revision 12
# baseline (speedup 1.0000x reference)
"""Block-diagonal complex matmul kernel for trn2 (8 NeuronCores).

Reference computation:
  xp = take(x, perm_idx, axis=-2).reshape(B, 2, M, S)
  y_re = xp_re @ hr1 + xp_im @ hi1   (per block a of M)
  y_im = xp_re @ hi2 + xp_im @ hr2
  out  = stack([y_re, y_im], 1).reshape(B, 2, N, R)

Sharding: block dim M=1024 split across 8 cores (128 blocks each).
Permutation gather + all layout shuffles happen host-side in numpy.

Device kernel (per core), per block a:
  psum[16, 256] = x_re[:, a].T @ [hr1[a] | hi2[a]]   (start)
                + x_im[:, a].T @ [hi1[a] | hr2[a]]   (stop)
  -> cols 0:128 = y_re[a], cols 128:256 = y_im[a]

Weights are fp8 e3m4 (scaled x16, with 1/16 folded into the fp16 x), which
halves HBM weight traffic vs fp16 while keeping rel-err ~1.3% (< 2e-2 gate).
8 blocks pack one PSUM bank [128, 512]: 4 row-tiles x 2 col-halves; a single
DVE copy drains the bank to fp16 SBUF per group.
"""

import os
import numpy as np
import ml_dtypes

B = 16
N = 4096
R = 32
M = 1024   # blocks
S = 128    # block size (contract dim)
NCORES = 8
MLOC = M // NCORES   # 128 blocks per core
GB = 8               # blocks per psum bank / weight DMA group
NGRP = MLOC // GB    # 16 groups
WSCALE = 16.0        # weight scale into e3m4 normal range (1/16 folded into x)

_NC_CACHE = {}


def _build_nc():
    import concourse.bacc as bacc
    import concourse.bass as bass
    import concourse.mybir as mybir
    from concourse import tile

    f16 = mybir.dt.float16
    f8 = mybir.dt.float8e3
    f32 = mybir.dt.float32
    nc = bacc.Bacc(None, target_bir_lowering=False)

    WC = 4 * S  # 512 fp8 cols per block: [hr1|hi2|hi1|hr2]
    xr = nc.dram_tensor("xr", [S, MLOC * B], f16, kind="ExternalInput")
    xi = nc.dram_tensor("xi", [S, MLOC * B], f16, kind="ExternalInput")
    w = nc.dram_tensor("w", [S, MLOC * WC], f8, kind="ExternalInput")
    # y rows: 16 batch; cols: group x row-tile x 512 (compacted)
    y = nc.dram_tensor("y", [B, NGRP * 4 * 2 * 2 * S], f16, kind="ExternalOutput")

    with tile.TileContext(nc) as tc:
        with (
            tc.tile_pool(name="xp", bufs=1) as xpool,
            tc.tile_pool(name="wp", bufs=6) as wpool,
            tc.tile_pool(name="op", bufs=6) as opool,
            tc.tile_pool(name="ps", bufs=1, space=bass.MemorySpace.PSUM) as ps,
        ):
            # x in two chunks so the first matmuls don't wait on the full x
            XC0 = 2 * GB * B  # first 2 groups of blocks
            xr_c0 = xpool.tile([S, XC0], f16, name="xr_c0")
            xi_c0 = xpool.tile([S, XC0], f16, name="xi_c0")
            xr_c1 = xpool.tile([S, MLOC * B - XC0], f16, name="xr_c1")
            xi_c1 = xpool.tile([S, MLOC * B - XC0], f16, name="xi_c1")
            # x on the scalar HWDGE ring so it overlaps the w loads (sync ring)
            nc.scalar.dma_start(xr_c0[:], xr[:, :XC0])
            nc.scalar.dma_start(xi_c0[:], xi[:, :XC0])
            nc.scalar.dma_start(xr_c1[:], xr[:, XC0:])
            nc.scalar.dma_start(xi_c1[:], xi[:, XC0:])

            # 6 static psum banks, zeroed once so junk rows are defined
            pts = [ps.tile([128, 2 * 2 * S], f32, name=f"pt{i}") for i in range(6)]
            for pt in pts:
                nc.vector.memset(pt[:], 0.0)

            # weight DMAs: one psum-group (512 KiB) per transfer, 6 in flight
            wts = {}
            for wg in range(NGRP):
                wt = wpool.tile([S, GB * WC], f8)
                nc.sync.dma_start(wt[:], w[:, wg * GB * WC:(wg + 1) * GB * WC])
                wts[wg] = wt

            for g in range(NGRP):
                wt = wts[g]
                pt = pts[g % 6]
                for i in range(GB):
                    a = g * GB + i
                    t, h = i % 4, i // 4
                    po = pt[32 * t:32 * t + B, 256 * h:256 * h + 256]
                    if a < 2 * GB:
                        xrs = xr_c0[:, a * B:(a + 1) * B]
                        xis = xi_c0[:, a * B:(a + 1) * B]
                    else:
                        xrs = xr_c1[:, (a - 2 * GB) * B:(a - 2 * GB + 1) * B]
                        xis = xi_c1[:, (a - 2 * GB) * B:(a - 2 * GB + 1) * B]
                    nc.tensor.matmul(po, xrs, wt[:, i * WC:i * WC + 256],
                                     start=True, stop=False,
                                     tile_position=(0, 32 * t))
                    nc.tensor.matmul(po, xis, wt[:, i * WC + 256:(i + 1) * WC],
                                     start=False, stop=True,
                                     tile_position=(0, 32 * t))
                # compact the 4 psum row-tiles side by side in columns
                # (engine APs need 32-aligned partition bases); DVE + ACT split
                ot = opool.tile([B, 4 * 2 * 2 * S], f16)
                nc.vector.tensor_scalar_mul(ot[:, 0:512], pt[0:16, :], 1.0)
                nc.vector.tensor_scalar_mul(ot[:, 512:1024], pt[32:48, :], 1.0)
                nc.scalar.copy(ot[:, 1024:1536], pt[64:80, :])
                nc.scalar.copy(ot[:, 1536:2048], pt[96:112, :])
                nc.gpsimd.dma_start(y[:, g * 2048:(g + 1) * 2048], ot[:])
    nc.compile()
    return nc


def kernel(x, hr1, hi1, hr2, hi2, perm_idx):
    from concourse.bass_utils import run_bass_kernel_spmd

    if "nc" not in _NC_CACHE:
        _NC_CACHE["nc"] = _build_nc()
    nc = _NC_CACHE["nc"]

    x = np.asarray(x, dtype=np.float32)
    perm_idx = np.asarray(perm_idx)
    # host-side permutation gather + regroup into M blocks of size S
    xp = x[:, :, perm_idx, :].reshape(B, 2, M, S)

    f8 = ml_dtypes.float8_e3m4
    in_maps = []
    for c in range(NCORES):
        sl = slice(c * MLOC, (c + 1) * MLOC)
        # [B, MLOC, S] -> [S(j), MLOC, B] -> [S, MLOC*B], scaled by 1/16
        xre = np.ascontiguousarray(
            np.transpose(xp[:, 0, sl, :], (2, 1, 0)) * (1.0 / WSCALE)
        ).astype(np.float16).reshape(S, MLOC * B)
        xim = np.ascontiguousarray(
            np.transpose(xp[:, 1, sl, :], (2, 1, 0)) * (1.0 / WSCALE)
        ).astype(np.float16).reshape(S, MLOC * B)
        # per block 512 cols: [W1 = hr1|hi2, W2 = hi1|hr2], e3m4 scaled x16
        wc = np.concatenate([hr1[sl], hi2[sl], hi1[sl], hr2[sl]], axis=2)
        wc = np.ascontiguousarray(np.transpose(wc, (1, 0, 2))).reshape(S, MLOC * 4 * S)
        wq = np.clip(wc * WSCALE, -15.5, 15.5).astype(f8)
        in_maps.append({"xr": xre, "xi": xim, "w": wq})

    trace = bool(os.environ.get("KERNEL_TRACE"))
    kwargs = {}
    if trace:
        kwargs["tmpdir"] = os.environ.get("KERNEL_TRACE_DIR") or None
    res = run_bass_kernel_spmd(nc, in_maps, core_ids=list(range(NCORES)), trace=trace, **kwargs)
    if trace and res.exec_time_ns is not None:
        print(f"HW exec time: {res.exec_time_ns} ns")
        _NC_CACHE["exec_time_ns"] = res.exec_time_ns
        _NC_CACHE["profile"] = res

    out = np.empty((B, 2, M, S), dtype=np.float32)
    for c in range(NCORES):
        a0 = c * MLOC
        yd = res.results[c]["y"].astype(np.float32)
        # rows: b(16); cols: g(16) x t(4) x h(2) x 256
        yv = yd.reshape(B, NGRP, 4, 2, 256)                 # [b, g, t, h, 256]
        yv = yv.transpose(0, 1, 3, 2, 4).reshape(B, MLOC, 256)  # a = 8g+4h+t
        out[:, 0, a0:a0 + MLOC, :] = yv[:, :, :S]
        out[:, 1, a0:a0 + MLOC, :] = yv[:, :, S:]
    return out.reshape(B, 2, N, R)


# revision 15
# speedup vs baseline: 1.0941x; 1.0941x over previous
"""Block-diagonal complex matmul kernel for trn2 (8 NeuronCores).

Reference computation:
  xp = take(x, perm_idx, axis=-2).reshape(B, 2, M, S)
  y_re = xp_re @ hr1 + xp_im @ hi1   (per block a of M)
  y_im = xp_re @ hi2 + xp_im @ hr2
  out  = stack([y_re, y_im], 1).reshape(B, 2, N, R)

Sharding: block dim M=1024 split across 8 cores (128 blocks each).
Permutation gather + all layout shuffles happen host-side in numpy.

Device kernel (per core), per block a:
  psum[16, 256] = x_re[:, a].T @ [hr1[a] | hi2[a]]   (start)
                + x_im[:, a].T @ [hi1[a] | hr2[a]]   (stop)
  -> cols 0:128 = y_re[a], cols 128:256 = y_im[a]

Weights are fp8 e3m4 (scaled x16, with 1/16 folded into the fp16 x), which
halves HBM weight traffic vs fp16 while keeping rel-err ~1.3% (< 2e-2 gate).
8 blocks pack one PSUM bank [128, 512]: 4 row-tiles x 2 col-halves; a single
DVE copy drains the bank to fp16 SBUF per group.
"""

import os
import numpy as np
import ml_dtypes

B = 16
N = 4096
R = 32
M = 1024   # blocks
S = 128    # block size (contract dim)
NCORES = 8
MLOC = M // NCORES   # 128 blocks per core
GB = 8               # blocks per psum bank / weight DMA group
NGRP = MLOC // GB    # 16 groups
WSCALE = 16.0        # weight scale into e3m4 normal range (1/16 folded into x)

_NC_CACHE = {}


def _build_nc():
    import concourse.bacc as bacc
    import concourse.bass as bass
    import concourse.mybir as mybir
    from concourse import tile

    f16 = mybir.dt.float16
    f8 = mybir.dt.float8e3
    f32 = mybir.dt.float32
    nc = bacc.Bacc(None, target_bir_lowering=False)

    WC = 4 * S  # 512 fp8 cols per block: [hr1|hi2|hi1|hr2]
    xr = nc.dram_tensor("xr", [S, MLOC * B], f16, kind="ExternalInput")
    xi = nc.dram_tensor("xi", [S, MLOC * B], f16, kind="ExternalInput")
    w = nc.dram_tensor("w", [S, MLOC * WC], f8, kind="ExternalInput")
    # y rows: 4 row-tiles x (16 data + 16 junk); cols: group * 512; fp8 e3m4
    y = nc.dram_tensor("y", [128, NGRP * 2 * 2 * S], f8, kind="ExternalOutput")

    with tile.TileContext(nc) as tc:
        with (
            tc.tile_pool(name="xp", bufs=1) as xpool,
            tc.tile_pool(name="wp", bufs=6) as wpool,
            tc.tile_pool(name="op", bufs=6) as opool,
            tc.tile_pool(name="ps", bufs=1, space=bass.MemorySpace.PSUM) as ps,
        ):
            # x in two chunks so the first matmuls don't wait on the full x
            XC0 = 2 * GB * B  # first 2 groups of blocks
            xr_c0 = xpool.tile([S, XC0], f16, name="xr_c0")
            xi_c0 = xpool.tile([S, XC0], f16, name="xi_c0")
            xr_c1 = xpool.tile([S, MLOC * B - XC0], f16, name="xr_c1")
            xi_c1 = xpool.tile([S, MLOC * B - XC0], f16, name="xi_c1")
            # x on the scalar HWDGE ring so it overlaps the w loads (sync ring)
            nc.scalar.dma_start(xr_c0[:], xr[:, :XC0])
            nc.scalar.dma_start(xi_c0[:], xi[:, :XC0])
            nc.scalar.dma_start(xr_c1[:], xr[:, XC0:])
            nc.scalar.dma_start(xi_c1[:], xi[:, XC0:])

            # 6 static psum banks, zeroed once so junk rows are defined
            pts = [ps.tile([128, 2 * 2 * S], f32, name=f"pt{i}") for i in range(6)]
            for pt in pts:
                nc.vector.memset(pt[:], 0.0)

            # weight DMAs: one psum-group (512 KiB) per transfer, 6 in flight
            wts = {}
            for wg in range(NGRP):
                wt = wpool.tile([S, GB * WC], f8)
                nc.sync.dma_start(wt[:], w[:, wg * GB * WC:(wg + 1) * GB * WC])
                wts[wg] = wt

            for g in range(NGRP):
                wt = wts[g]
                pt = pts[g % 6]
                for i in range(GB):
                    a = g * GB + i
                    t, h = i % 4, i // 4
                    po = pt[32 * t:32 * t + B, 256 * h:256 * h + 256]
                    if a < 2 * GB:
                        xrs = xr_c0[:, a * B:(a + 1) * B]
                        xis = xi_c0[:, a * B:(a + 1) * B]
                    else:
                        xrs = xr_c1[:, (a - 2 * GB) * B:(a - 2 * GB + 1) * B]
                        xis = xi_c1[:, (a - 2 * GB) * B:(a - 2 * GB + 1) * B]
                    nc.tensor.matmul(po, xrs, wt[:, i * WC:i * WC + 256],
                                     start=True, stop=False,
                                     tile_position=(0, 32 * t))
                    nc.tensor.matmul(po, xis, wt[:, i * WC + 256:(i + 1) * WC],
                                     start=False, stop=True,
                                     tile_position=(0, 32 * t))
                # one full-width copy (junk rows included), fp32 -> e3m4
                ot = opool.tile([128, 2 * 2 * S], f8)
                nc.vector.tensor_scalar_mul(ot[:], pt[:], 1.0)
                nc.gpsimd.dma_start(y[:, g * 512:(g + 1) * 512], ot[:])
    nc.compile()
    return nc


def kernel(x, hr1, hi1, hr2, hi2, perm_idx):
    from concourse.bass_utils import run_bass_kernel_spmd

    if "nc" not in _NC_CACHE:
        _NC_CACHE["nc"] = _build_nc()
    nc = _NC_CACHE["nc"]

    x = np.asarray(x, dtype=np.float32)
    perm_idx = np.asarray(perm_idx)
    # host-side permutation gather + regroup into M blocks of size S
    xp = x[:, :, perm_idx, :].reshape(B, 2, M, S)

    f8 = ml_dtypes.float8_e3m4
    in_maps = []
    for c in range(NCORES):
        sl = slice(c * MLOC, (c + 1) * MLOC)
        # [B, MLOC, S] -> [S(j), MLOC, B] -> [S, MLOC*B], scaled by 1/16
        xre = np.ascontiguousarray(
            np.transpose(xp[:, 0, sl, :], (2, 1, 0)) * (1.0 / WSCALE)
        ).astype(np.float16).reshape(S, MLOC * B)
        xim = np.ascontiguousarray(
            np.transpose(xp[:, 1, sl, :], (2, 1, 0)) * (1.0 / WSCALE)
        ).astype(np.float16).reshape(S, MLOC * B)
        # per block 512 cols: [W1 = hr1|hi2, W2 = hi1|hr2], e3m4 scaled x16
        wc = np.concatenate([hr1[sl], hi2[sl], hi1[sl], hr2[sl]], axis=2)
        wc = np.ascontiguousarray(np.transpose(wc, (1, 0, 2))).reshape(S, MLOC * 4 * S)
        wq = np.clip(wc * WSCALE, -15.5, 15.5).astype(f8)
        in_maps.append({"xr": xre, "xi": xim, "w": wq})

    trace = bool(os.environ.get("KERNEL_TRACE"))
    kwargs = {}
    if trace:
        kwargs["tmpdir"] = os.environ.get("KERNEL_TRACE_DIR") or None
    res = run_bass_kernel_spmd(nc, in_maps, core_ids=list(range(NCORES)), trace=trace, **kwargs)
    if trace and res.exec_time_ns is not None:
        print(f"HW exec time: {res.exec_time_ns} ns")
        _NC_CACHE["exec_time_ns"] = res.exec_time_ns
        _NC_CACHE["profile"] = res

    out = np.empty((B, 2, M, S), dtype=np.float32)
    for c in range(NCORES):
        a0 = c * MLOC
        yd = res.results[c]["y"].astype(np.float32)
        # rows: t(4) x [16 data + 16 junk]; cols: g(16) x [h(2) x 256]
        yv = yd.reshape(4, 32, NGRP, 512)[:, :B]            # [t, b, g, 512]
        yv = yv.reshape(4, B, NGRP, 2, 256)                 # [t, b, g, h, 256]
        yv = yv.transpose(1, 2, 3, 0, 4).reshape(B, MLOC, 256)  # a = 8g+4h+t
        out[:, 0, a0:a0 + MLOC, :] = yv[:, :, :S]
        out[:, 1, a0:a0 + MLOC, :] = yv[:, :, S:]
    return out.reshape(B, 2, N, R)
